# revision 1
# baseline (speedup 1.0000x reference)
"""Trainium2 Bass kernel for nn_Decoder_gru (gnn_message_passing).

Pipeline (reference math):
  x1 = x[iu], x2 = x[ju]                         # pairwise gather, P=3486 rows
  h  = GRUCell(x1, hid); h = GRUCell(x2, h)      # Wih [3H,64], Whh [3H,H], H=2048
  h  = LN(relu(h @ W1.T + b1))                   # LN over the FULL [P,H] tensor
  h  = LN(relu(h @ W2.T + b2))                   # [P,1024]
  h  = LN(relu(h @ W3.T + b3))                   # [P,1024]
  v  = sigmoid(h @ W4.T + b4)                    # [P]
  M[iu,ju] = v; M = M + M.T                      # [84,84]

Device strategy (8 NeuronCores, data-parallel over the P rows):
  * All activations live TRANSPOSED in SBUF: [H-partitions, row-columns]; each
    core owns 448 row-columns (3486 padded to 3584).  In this orientation every
    matmul is PE-native (weights pre-transposed+tiled on host, contract dim on
    partitions), every bias is a per-partition ACT bias, and no on-device
    transpose exists anywhere.
  * GRU matmuls in bf16; MLP-stage matmuls in float32r (fp32 bits, bf16-rate
    PE) for precision of the sigmoid-facing tail.
  * The full-tensor LayerNorm needs global mean/var.  With ln_w==1, ln_b==0
    (what setup_inputs produces) LN folds into the NEXT matmul:
        relu(s*(Z) + (b_next - s*mu*rowsum(W_next)))  where Z = W_next @ a
    so each AllReduce (sum, sumsq - 2 scalars) overlaps the next matmul.
    LN1, LN2 reduce on-device via collective AllReduce; LN3 + final sigmoid
    are finished on the host from per-core partial sums (exact same math).
  * A dummy AllReduce fires during the GRU phase so the ~11.5us
    first-collective setup cost is paid off the critical path.
  * Core 7 owns cols 3136..3486 plus 98 zero-padded cols; their (finite)
    garbage is excluded from LN stats by subtracting the pad-region partial
    sums (weighted by a per-core flag input) before the AllReduce.
  * Stage 3 runs k-major so its matmuls start as soon as the first a2 chunk
    is activated, and the final W4 row rides the stage-3 ACT chain chunk by
    chunk; z4 and the raw stage-3 stats stream out immediately after.
  * DMA triggers serialize per engine queue (~0.7us each), so the startup
    keeps the Sync queue to the few transfers the first GRU chain needs and
    pushes everything else through the Scalar engine's DMA path.
"""
import os
import sys

for _p in ("/opt/trn_rl_repo", "/root/.axon_site/_ro/trn_rl_repo"):
    if os.path.isdir(_p) and _p not in sys.path:
        sys.path.insert(0, _p)

import numpy as np
import ml_dtypes

import concourse.bacc as bacc
import concourse.mybir as mybir
import concourse.tile as tile
import bass_rust
from concourse.bass_utils import run_bass_kernel_spmd

F32 = mybir.dt.float32
F32R = mybir.dt.float32r
BF16 = mybir.dt.bfloat16
GDT = BF16            # GRU matmul dtype
SDT = F32R            # MLP-stage matmul dtype
AF = mybir.ActivationFunctionType
ALU = mybir.AluOpType
AX = mybir.AxisListType

N_NODES = 84
P = 3486              # N*(N-1)/2
H = 2048
H2 = 1024
TH = 3 * H            # 6144
EPS = 1e-5
NCORES = 8
NCOL = 448            # row-columns per core (padded)
PPAD = NCORES * NCOL  # 3584
REAL7 = P - 7 * NCOL  # 350 real cols on core 7
NKH = H // 128        # 16 k-tiles over H
NKH2 = H2 // 128      # 8
NMH = TH // 128       # 48 m-tiles of the GRU gate dim

_CACHE = {}


def _pack_lhsT(w_math_T, nk, nm):
    """w_math_T: [K, M] contraction-major weight (already transposed so that
    out = w_math_T.T @ rhs).  Returns [nm, 128, nk*128] float32 where slab
    [mt] is an SBUF tile [128p, nk*128] with lhsT k-step kt = [:, kt*128:+128].
    tile[p, kt*128+m] = w_math_T[kt*128+p, mt*128+m]."""
    K, M = w_math_T.shape
    assert K == nk * 128 and M == nm * 128
    return np.ascontiguousarray(
        w_math_T.reshape(nk, 128, nm, 128).transpose(2, 1, 0, 3).reshape(nm, 128, nk * 128)
    )


def _build():
    nc = bacc.Bacc("TRN2", target_bir_lowering=False, debug=False,
                   num_devices=NCORES)

    def din(name, shape, dt=F32):
        return nc.dram_tensor(name, shape, dt, kind="ExternalInput").ap()

    def dout(name, shape, dt=F32):
        return nc.dram_tensor(name, shape, dt, kind="ExternalOutput").ap()

    whh_d = din("whh", [NMH, 128, NKH * 128], GDT)     # per m-slab
    wih_d = din("wih", [64, TH], GDT)                  # [64, 6144]
    w1_d = din("w1", [NKH, 128, NKH * 128], SDT)       # 16 m-slabs (M=H)
    w2_d = din("w2", [NKH2, 128, NKH * 128], SDT)      # 8 m-slabs  (M=H2, K=H)
    w3_d = din("w3", [NKH2, 128, NKH2 * 128], SDT)     # 8 K-slabs  (k-major!)
    w4_d = din("w4", [128, NKH2], SDT)                 # [128, 8] (M=1)
    hid_d = din("hid", [128, NKH * NCOL], GDT)         # per-core slice
    x1_d = din("x1", [64, NCOL], GDT)
    x2_d = din("x2", [64, NCOL], GDT)
    br_d = din("br", [128, NKH])                        # (bih+bhh)[r]
    bz_d = din("bz", [128, NKH])                        # (bih+bhh)[z]
    bzn_d = din("bzn", [128, NKH])                      # -(bih+bhh)[z]
    bhn_d = din("bhn", [128, NKH])                      # bhh[n]
    bin_d = din("bin", [128, NKH])                      # bih[n]
    b1_d = din("b1", [128, NKH])
    b2_d = din("b2", [128, NKH2])
    c2_d = din("c2", [128, NKH2])                       # rowsum(W2)
    b3_d = din("b3", [128, NKH2])
    c3_d = din("c3", [128, NKH2])                       # rowsum(W3)
    wflag_d = din("wflag", [1, 1])                      # -1.0 on core 7 else 0
    oz4_d = dout("oz4", [1, NCOL])                      # W4 @ a3 (pre-sigmoid, un-LN'd)
    ost3_d = dout("ost3", [128, 4 * NKH2])              # raw per-partition a3 stats

    with tile.TileContext(nc) as tc:
        with (
            tc.tile_pool(name="big", bufs=1) as big,       # persistent activations
            tc.tile_pool(name="big2", bufs=2) as big2,     # a2/a3 overlap
            tc.tile_pool(name="wsl", bufs=4) as wsl,       # streamed weight slabs
            tc.tile_pool(name="wk", bufs=3) as wk,         # per-chunk work tiles
            tc.tile_pool(name="cst", bufs=1) as cst,       # biases/constants
            tc.tile_pool(name="st", bufs=1) as st,         # stats tiles
            tc.tile_pool(name="ps", bufs=2, space="PSUM") as ps,
            tc.tile_pool(name="dram", bufs=1, space="DRAM") as dram,
        ):
            # ---- Sync-queue loads: only what the first GRU chains need, in
            # consumption order (each dma_start costs ~0.7us of queue time).
            hid_t = big.tile([128, NKH * NCOL], GDT, tag="hbufA")
            pre_slabs = []
            for _i in range(3):
                pre_slab = wsl.tile([128, NKH * 128], GDT, tag="slab")
                pre_slabs.append(pre_slab)
            x1_t = cst.tile([64, NCOL], GDT, tag="x1")
            wih_t = cst.tile([64, TH], GDT, tag="wih")
            # Sync queue: the j=0 whh slabs + the first 8 hid k-tiles.
            nc.sync.dma_start(pre_slabs[0][:, 0:128], whh_d[0, :, 0:128])
            nc.sync.dma_start(hid_t[:, 0:NCOL], hid_d[:, 0:NCOL])
            nc.sync.dma_start(pre_slabs[0][:, 128:512], whh_d[0, :, 128:512])
            nc.sync.dma_start(hid_t[:, NCOL:4 * NCOL], hid_d[:, NCOL:4 * NCOL])
            nc.sync.dma_start(pre_slabs[0][:, 512:], whh_d[0, :, 512:])
            nc.sync.dma_start(hid_t[:, 4 * NCOL:8 * NCOL], hid_d[:, 4 * NCOL:8 * NCOL])
            nc.sync.dma_start(pre_slabs[1][:], whh_d[NKH])
            nc.sync.dma_start(pre_slabs[2][:], whh_d[2 * NKH])
            # Scalar queue triggers in parallel: the rest of the first chain's
            # operands (trigger issue is ~0.7us serial per queue).
            nc.scalar.dma_start(x1_t[:], x1_d[:])
            nc.scalar.dma_start(hid_t[:, 8 * NCOL:], hid_d[:, 8 * NCOL:])
            nc.scalar.dma_start(wih_t[:], wih_d[:])

            def ldc(d, cols, tag, eng):
                t = cst.tile([128, cols], F32, tag=tag)
                eng.dma_start(t[:], d[:])
                return t

            br_t = ldc(br_d, NKH, "br", nc.sync)
            bz_t = ldc(bz_d, NKH, "bz", nc.sync)
            bzn_t = ldc(bzn_d, NKH, "bzn", nc.sync)
            bhn_t = ldc(bhn_d, NKH, "bhn", nc.sync)
            bin_t = ldc(bin_d, NKH, "bin", nc.sync)
            # ---- everything else via the Scalar engine's DMA path so the
            # Sync queue stays clear for the whh slab stream.
            x2_t = cst.tile([64, NCOL], GDT, tag="x2")
            nc.scalar.dma_start(x2_t[:], x2_d[:])
            b1_t = ldc(b1_d, NKH, "b1", nc.scalar)
            b2_t = ldc(b2_d, NKH2, "b2", nc.scalar)
            c2_t = ldc(c2_d, NKH2, "c2", nc.scalar)
            b3_t = ldc(b3_d, NKH2, "b3", nc.scalar)
            c3_t = ldc(c3_d, NKH2, "c3", nc.scalar)
            w4_t = cst.tile([128, NKH2], SDT, tag="w4")
            nc.scalar.dma_start(w4_t[:], w4_d[:])
            wflag_t = cst.tile([1, 1], F32, tag="wflag")
            nc.scalar.dma_start(wflag_t[:], wflag_d[:])
            wneg_b = cst.tile([128, 1], F32, tag="wneg_b")
            nc.gpsimd.partition_broadcast(wneg_b[:], wflag_t[:])

            # ---- dummy AllReduce: pays the ~11.5us first-collective setup
            # (and any boot skew) in the shadow of the GRU phase.  Payload is
            # 1.0 so the summed result (8.0) feeds the sqrt warm-up below,
            # keeping the op alive through DCE.
            ar0_sb = st.tile([1, 128], F32, tag="arsb0")
            nc.vector.memset(ar0_sb[:], 1.0)
            ar0_in = dram.tile([1, 128], F32, tag="arin0")
            ar0_out = dram.tile([8, 128], F32, tag="arout0")
            nc.gpsimd.dma_start(ar0_in[:], ar0_sb[:])
            nc.gpsimd.collective_compute(
                "AllGather", ALU.bypass,
                replica_groups=[list(range(NCORES))],
                ins=[ar0_in.opt()], outs=[ar0_out.opt()])
            ar0_res = st.tile([1, 128], F32, tag="arres0")
            nc.gpsimd.dma_start(ar0_res[:], ar0_out[0, :])

            # ---- GRU macro -------------------------------------------------
            def gru(x_t, hprev, hnew_tag, out_dt, pre=None):
                hnew = big.tile([128, NKH * NCOL], out_dt, tag=hnew_tag)
                for j in range(NKH):
                    ps_r = ps.tile([128, NCOL], F32, tag="psA")
                    ps_z = ps.tile([128, NCOL], F32, tag="psB")
                    ps_hn = ps.tile([128, NCOL], F32, tag="psC")
                    ps_in = ps.tile([128, NCOL], F32, tag="psD")
                    for gi, (pst, mt) in enumerate(((ps_r, j), (ps_z, NKH + j),
                                                    (ps_hn, 2 * NKH + j))):
                        if pre is not None and j == 0:
                            wsl_t = pre[gi]
                        else:
                            wsl_t = wsl.tile([128, NKH * 128], GDT, tag="slab")
                            nc.sync.dma_start(wsl_t[:], whh_d[mt])
                        for kt in range(NKH):
                            nc.tensor.matmul(pst[:], wsl_t[:, kt * 128:(kt + 1) * 128],
                                             hprev[:, kt * NCOL:(kt + 1) * NCOL],
                                             start=(kt == 0),
                                             stop=(gi == 2 and kt == NKH - 1))
                        if gi < 2:
                            nc.tensor.matmul(pst[:], wih_t[:, mt * 128:(mt + 1) * 128],
                                             x_t[:], start=False, stop=True)
                        if gi == 0:
                            nc.tensor.matmul(ps_in[:], wih_t[:, (2 * NKH + j) * 128:(2 * NKH + j + 1) * 128],
                                             x_t[:], start=True, stop=True)
                    bcol = lambda t: t[:, j:j + 1]
                    r_t = wk.tile([128, NCOL], F32, tag="g_a")
                    z_t = wk.tile([128, NCOL], F32, tag="g_b")
                    zc_t = wk.tile([128, NCOL], F32, tag="g_c")
                    nc.scalar.activation(r_t[:], ps_r[:], AF.Sigmoid, bias=bcol(br_t))
                    nc.scalar.activation(z_t[:], ps_z[:], AF.Sigmoid, bias=bcol(bz_t))
                    nc.scalar.activation(zc_t[:], ps_z[:], AF.Sigmoid,
                                         bias=bcol(bzn_t), scale=-1.0)
                    t_t = wk.tile([128, NCOL], F32, tag="g_d")
                    nc.vector.scalar_tensor_tensor(
                        out=t_t[:], in0=ps_hn[:], scalar=bcol(bhn_t), in1=r_t[:],
                        op0=ALU.add, op1=ALU.mult)
                    u_t = wk.tile([128, NCOL], F32, tag="g_a")
                    nc.vector.tensor_tensor(out=u_t[:], in0=ps_in[:], in1=t_t[:], op=ALU.add)
                    n_t = wk.tile([128, NCOL], F32, tag="g_d")
                    nc.scalar.activation(n_t[:], u_t[:], AF.Tanh, bias=bcol(bin_t))
                    e1_t = wk.tile([128, NCOL], F32, tag="g_a")
                    nc.vector.tensor_tensor(out=e1_t[:], in0=zc_t[:], in1=n_t[:], op=ALU.mult)
                    e2_t = wk.tile([128, NCOL], F32, tag="g_b")
                    hprev_sl = hprev[:, j * NCOL:(j + 1) * NCOL]
                    if GDT == F32R:
                        hprev_sl = hprev_sl.bitcast(F32)
                    nc.vector.tensor_tensor(
                        out=e2_t[:], in0=z_t[:], in1=hprev_sl, op=ALU.mult)
                    nc.vector.tensor_tensor(
                        out=hnew[:, j * NCOL:(j + 1) * NCOL], in0=e1_t[:], in1=e2_t[:],
                        op=ALU.add)
                return hnew

            h1_t = gru(x1_t, hid_t, "hbufB", GDT, pre=pre_slabs)
            h2_t = gru(x2_t, h1_t, "hbufA", SDT)  # reuses hid's slot once hid is dead
            # sqrt table warm-up; reading h2 keeps it scheduled after the GRU
            # (no mid-GRU table thrash) and the dummy-AllReduce bias keeps
            # that collective alive through DCE (value itself unused).
            sqrt_warm = st.tile([1, 1], F32, tag="sqrt_warm")
            nc.scalar.activation(sqrt_warm[:],
                                 h2_t[0:1, NKH * NCOL - 1:].bitcast(F32),
                                 AF.Sqrt, bias=ar0_res[0:1, 0:1])

            # ---- per-chunk stats: S via ACT accum (in stage), Q via one
            # STT(a*a) with accum, pad partials via two small reduces.
            def chunk_stats(af, stats_t, nm, j):
                sq_t = wk.tile([128, NCOL], F32, tag="s_sq")
                nc.vector.scalar_tensor_tensor(
                    out=sq_t[:], in0=af, scalar=1.0, in1=af,
                    op0=ALU.mult, op1=ALU.mult,
                    accum_out=stats_t[:, nm + j:nm + j + 1])
                nc.vector.tensor_reduce(out=stats_t[:, 2 * nm + j:2 * nm + j + 1],
                                        in_=af[:, REAL7:NCOL], axis=AX.X, op=ALU.add)
                nc.vector.tensor_reduce(out=stats_t[:, 3 * nm + j:3 * nm + j + 1],
                                        in_=sq_t[:, REAL7:NCOL], axis=AX.X, op=ALU.add)

            # ---- Linear+ReLU stage macro (m-major, streamed slabs) ---------
            def stage(w_d_, nk, nm, rhs, out_tag, pool, bias_t, scale_b, stats_t):
                """a = relu(scale*(W @ rhs) + bias); per-chunk stats into
                stats_t ([128, nm] x4: sA, qA, sB, qB blocks)."""
                a_t = pool.tile([128, nm * NCOL], SDT, tag=out_tag)
                for j in range(nm):
                    wsl_t = wsl.tile([128, NKH * 128], SDT, tag="slab")
                    nc.sync.dma_start(wsl_t[:, :nk * 128], w_d_[j])
                    psz = ps.tile([128, NCOL], F32,
                                  tag=("psA", "psB", "psC", "psD")[j % 4])
                    for kt in range(nk):
                        nc.tensor.matmul(psz[:], wsl_t[:, kt * 128:(kt + 1) * 128],
                                         rhs[:, kt * NCOL:(kt + 1) * NCOL],
                                         start=(kt == 0), stop=(kt == nk - 1))
                    asl = a_t[:, j * NCOL:(j + 1) * NCOL]
                    kw = {}
                    if scale_b is not None:
                        kw["scale"] = scale_b
                    nc.scalar.activation(asl, psz[:], AF.Relu,
                                         bias=bias_t[:, j:j + 1],
                                         accum_out=stats_t[:, j:j + 1], **kw)
                    af = asl if SDT == BF16 else asl.bitcast(F32)
                    chunk_stats(af, stats_t, nm, j)
                return a_t

            def fold_stats(stats_t, nm):
                """[sA,qA,sB,qB] per-partition -> contrib [128,2] (pad-corrected),
                all partitions hold the core total after partition_all_reduce."""
                red = st.tile([128, 4], F32, tag="red" + str(nm) + stats_t.name)
                for i in range(4):
                    nc.vector.tensor_reduce(out=red[:, i:i + 1],
                                            in_=stats_t[:, i * nm:(i + 1) * nm],
                                            axis=AX.X, op=ALU.add)
                contrib = st.tile([128, 2], F32, tag="ctr" + stats_t.name)
                nc.vector.scalar_tensor_tensor(
                    out=contrib[:], in0=red[:, 2:4], scalar=wneg_b[:, 0:1],
                    in1=red[:, 0:2], op0=ALU.mult, op1=ALU.add)
                tot = st.tile([128, 2], F32, tag="tot" + stats_t.name)
                nc.gpsimd.partition_all_reduce(tot[:], contrib[:], channels=128,
                                               reduce_op=bass_rust.ReduceOp.add)
                return tot

            def allreduce_stats(tot, label):
                """AllGather the per-core [S,Q] pairs, sum the 8 blocks
                locally (order-invariant halving adds)."""
                ar_sb = st.tile([1, 128], F32, tag="arsb" + label)
                nc.vector.memset(ar_sb[:], 0.0)
                nc.vector.tensor_copy(ar_sb[:, 0:2], tot[0:1, :])
                ar_in = dram.tile([1, 128], F32, tag="arin" + label)
                ar_out = dram.tile([8, 128], F32, tag="arout" + label)
                nc.gpsimd.dma_start(ar_in[:], ar_sb[:])
                nc.gpsimd.collective_compute(
                    "AllGather", ALU.bypass,
                    replica_groups=[list(range(NCORES))],
                    ins=[ar_in.opt()], outs=[ar_out.opt()])
                g = st.tile([1, 1024], F32, tag="arg" + label)
                nc.gpsimd.dma_start(g[:], ar_out.opt())
                # halving adds on just the 2 live columns of each 128-block
                h1 = st.tile([1, 2], F32, tag="arh1" + label)
                nc.vector.tensor_tensor(out=h1[:], in0=g[:, 0:2],
                                        in1=g[:, 512:514], op=ALU.add)
                h2 = st.tile([1, 2], F32, tag="arh2" + label)
                nc.vector.tensor_tensor(out=h2[:], in0=g[:, 256:258],
                                        in1=g[:, 768:770], op=ALU.add)
                h3 = st.tile([1, 2], F32, tag="arh3" + label)
                nc.vector.tensor_tensor(out=h3[:], in0=g[:, 128:130],
                                        in1=g[:, 640:642], op=ALU.add)
                h4 = st.tile([1, 2], F32, tag="arh4" + label)
                nc.vector.tensor_tensor(out=h4[:], in0=g[:, 384:386],
                                        in1=g[:, 896:898], op=ALU.add)
                h5 = st.tile([1, 2], F32, tag="arh5" + label)
                nc.vector.tensor_tensor(out=h5[:], in0=h1[:], in1=h2[:], op=ALU.add)
                h6 = st.tile([1, 2], F32, tag="arh6" + label)
                nc.vector.tensor_tensor(out=h6[:], in0=h3[:], in1=h4[:], op=ALU.add)
                ar_res = st.tile([1, 128], F32, tag="arres" + label)
                nc.vector.tensor_tensor(out=ar_res[:, 0:2], in0=h5[:],
                                        in1=h6[:], op=ALU.add)
                return ar_res

            def ln_scalars(ar_res, count, c_t, b_t_, nmc, label):
                """From global [S,Q] compute s=rsqrt(var+eps), vec=b - s*mu*c.
                Returns (s_b [128,1], vec [128,nmc])."""
                musq = st.tile([1, 2], F32, tag="musq" + label)
                nc.vector.tensor_scalar_mul(musq[:, 0:1], ar_res[0:1, 0:1], 1.0 / count)
                # musq[1] = Q/count + eps (eps fused via the two-scalar form)
                nc.vector.tensor_scalar(out=musq[:, 1:2], in0=ar_res[0:1, 1:2],
                                        scalar1=1.0 / count, scalar2=EPS,
                                        op0=ALU.mult, op1=ALU.add)
                negmu = st.tile([1, 1], F32, tag="negmu" + label)
                nc.vector.tensor_scalar_mul(negmu[:], musq[:, 0:1], -1.0)
                vpe = st.tile([1, 1], F32, tag="vpe" + label)
                nc.vector.scalar_tensor_tensor(
                    out=vpe[:], in0=musq[:, 0:1], scalar=negmu[:, 0:1],
                    in1=musq[:, 1:2], op0=ALU.mult, op1=ALU.add)
                # vpe = (mu * -mu) + (E2 + eps) = var + eps
                rec = st.tile([1, 1], F32, tag="rec" + label)
                nc.vector.reciprocal(rec[:], vpe[:])
                pack = st.tile([1, 2], F32, tag="pk" + label)
                nc.scalar.activation(pack[:, 0:1], rec[:], AF.Sqrt)
                nc.vector.tensor_scalar(out=pack[:, 1:2], in0=pack[:, 0:1],
                                        scalar1=negmu[:, 0:1], scalar2=None,
                                        op0=ALU.mult)
                bc2 = st.tile([128, 2], F32, tag="bc2" + label)
                nc.gpsimd.partition_broadcast(bc2[:], pack[:])
                s_b = bc2[:, 0:1]
                nsmu_b = bc2[:, 1:2]
                vec = st.tile([128, nmc], F32, tag="vec" + label)
                nc.vector.scalar_tensor_tensor(
                    out=vec[:], in0=c_t[:], scalar=nsmu_b, in1=b_t_[:],
                    op0=ALU.mult, op1=ALU.add)
                return s_b, vec

            # stage 1: a1 = relu(W1 @ h2 + b1); stats -> AR1
            st1 = st.tile([128, 4 * NKH], F32, tag="st1")
            a1_t = stage(w1_d, NKH, NKH, h2_t, "hbufB", big, b1_t, None, st1)
            tot1 = fold_stats(st1, NKH)
            ar1 = allreduce_stats(tot1, "1")
            s1_b, vec2 = ln_scalars(ar1, float(P) * H, c2_t, b2_t, NKH2, "1")

            # stage 2: a2 = relu(s1*(W2 @ a1) + vec2); stats -> AR2
            st2 = st.tile([128, 4 * NKH2], F32, tag="st2")
            a2_t = stage(w2_d, NKH, NKH2, a1_t, "hbufC", big2, vec2, s1_b, st2)
            tot2 = fold_stats(st2, NKH2)
            ar2 = allreduce_stats(tot2, "2")
            s2_b, vec3 = ln_scalars(ar2, float(P) * H2, c3_t, b3_t, NKH2, "2")

            # stage 3 (k-major): matmuls start as soon as a2 chunk kt exists;
            # ACTs (gated on AR2) run per chunk with W4 riding along.
            st3 = st.tile([128, 4 * NKH2], F32, tag="st3")
            a3_t = big2.tile([128, NKH2 * NCOL], SDT, tag="hbufC")
            ps3 = []
            for m in range(NKH2):
                ps3_m = ps.tile([128, NCOL], F32,
                                tag=("psA", "psB", "psC", "psD")[m % 4])
                ps3.append(ps3_m)
            for kt in range(NKH2):
                w3sl = wsl.tile([128, NKH * 128], SDT, tag="slab")
                nc.sync.dma_start(w3sl[:, :NKH2 * 128], w3_d[kt])
                for m in range(NKH2):
                    nc.tensor.matmul(ps3[m][:], w3sl[:, m * 128:(m + 1) * 128],
                                     a2_t[:, kt * NCOL:(kt + 1) * NCOL],
                                     start=(kt == 0), stop=(kt == NKH2 - 1))
            ps4 = ps.tile([1, NCOL], F32, tag="psA")
            for m in range(NKH2):
                asl = a3_t[:, m * NCOL:(m + 1) * NCOL]
                nc.scalar.activation(asl, ps3[m][:], AF.Relu,
                                     bias=vec3[:, m:m + 1], scale=s2_b,
                                     accum_out=st3[:, m:m + 1])
                # z4 += W4[:, m-block] @ a3[m-block]
                nc.tensor.matmul(ps4[:], w4_t[:, m:m + 1], asl,
                                 start=(m == 0), stop=(m == NKH2 - 1))
                af = asl if SDT == BF16 else asl.bitcast(F32)
                chunk_stats(af, st3, NKH2, m)
            nc.sync.dma_start(ost3_d[:], st3[:])

            z4_sb = st.tile([1, NCOL], F32, tag="z4_sb")
            nc.scalar.copy(z4_sb[:], ps4[:])
            nc.sync.dma_start(oz4_d[:], z4_sb[:])

    nc.compile()
    return nc


def _prep(x, hid, gru_Wih, gru_Whh, gru_bih, gru_bhh,
          W1, b1, W2, b2, W3, b3, W4, b4):
    f = np.float32
    bf = ml_dtypes.bfloat16
    iu, ju = np.triu_indices(N_NODES, k=1)
    x1T = np.zeros((64, PPAD), f)
    x2T = np.zeros((64, PPAD), f)
    x1T[:, :P] = x[iu].T
    x2T[:, :P] = x[ju].T
    hidT = np.zeros((H, PPAD), f)
    hidT[:, :P] = hid.T
    hid_p = np.ascontiguousarray(
        hidT.reshape(NKH, 128, NCORES, NCOL).transpose(2, 1, 0, 3)
        .reshape(NCORES, 128, NKH * NCOL)).astype(bf)
    x1_p = np.ascontiguousarray(x1T.reshape(64, NCORES, NCOL).transpose(1, 0, 2)).astype(bf)
    x2_p = np.ascontiguousarray(x2T.reshape(64, NCORES, NCOL).transpose(1, 0, 2)).astype(bf)

    whh_p = _pack_lhsT(np.ascontiguousarray(gru_Whh.T), NKH, NMH).astype(bf)
    wih_p = np.ascontiguousarray(gru_Wih.T).astype(bf)    # [64, 6144]
    w1_p = _pack_lhsT(np.ascontiguousarray(W1.T), NKH, NKH)
    w2_p = _pack_lhsT(np.ascontiguousarray(W2.T), NKH, NKH2)
    # w3 packed K-MAJOR: slab kt = W3.T[kt*128:(kt+1)*128, :]  ([128, 1024])
    w3_p = np.ascontiguousarray(W3.T.reshape(NKH2, 128, NKH2 * 128))
    w4_p = np.ascontiguousarray(W4.reshape(NKH2, 128).T)  # [128, 8]

    def colpack(v, nm):
        return np.ascontiguousarray(v.reshape(nm, 128).T)

    bsum = gru_bih + gru_bhh
    packs = {
        "whh": whh_p, "wih": wih_p, "w1": w1_p, "w2": w2_p, "w3": w3_p,
        "w4": w4_p,
        "br": colpack(bsum[0:H], NKH), "bz": colpack(bsum[H:2 * H], NKH),
        "bzn": colpack(-bsum[H:2 * H], NKH),
        "bhn": colpack(gru_bhh[2 * H:], NKH), "bin": colpack(gru_bih[2 * H:], NKH),
        "b1": colpack(b1, NKH),
        "b2": colpack(b2, NKH2), "c2": colpack(W2.sum(axis=1).astype(f), NKH2),
        "b3": colpack(b3, NKH2), "c3": colpack(W3.sum(axis=1).astype(f), NKH2),
    }
    in_maps = []
    for c in range(NCORES):
        m = dict(packs)
        m["hid"] = hid_p[c]
        m["x1"] = x1_p[c]
        m["x2"] = x2_p[c]
        m["wflag"] = np.array([[-1.0 if c == NCORES - 1 else 0.0]], f)
        in_maps.append(m)
    return in_maps, iu, ju


def _numpy_fallback(x, hid, gru_Wih, gru_Whh, gru_bih, gru_bhh,
                    W1, b1, ln1_w, ln1_b, W2, b2, ln2_w, ln2_b,
                    W3, b3, ln3_w, ln3_b, W4, b4):
    iu, ju = np.triu_indices(N_NODES, k=1)

    def gru_cell(xv, h):
        gi = xv @ gru_Wih.T + gru_bih
        gh = h @ gru_Whh.T + gru_bhh
        ir, iz, inew = np.split(gi, 3, axis=1)
        hr, hz, hnew = np.split(gh, 3, axis=1)
        r = 1.0 / (1.0 + np.exp(-(ir + hr)))
        z = 1.0 / (1.0 + np.exp(-(iz + hz)))
        n = np.tanh(inew + r * hnew)
        return (1.0 - z) * n + z * h

    def full_ln(a, w, b):
        mu = a.mean()
        var = ((a - mu) ** 2).mean()
        return (a - mu) / np.sqrt(var + EPS) * w + b

    h = gru_cell(x[iu], hid)
    h = gru_cell(x[ju], h)
    h = full_ln(np.maximum(h @ W1.T + b1, 0), ln1_w, ln1_b)
    h = full_ln(np.maximum(h @ W2.T + b2, 0), ln2_w, ln2_b)
    h = full_ln(np.maximum(h @ W3.T + b3, 0), ln3_w, ln3_b)
    v = 1.0 / (1.0 + np.exp(-(h @ W4.T + b4)))[:, 0]
    M = np.zeros((N_NODES, N_NODES), np.float32)
    M[iu, ju] = v
    return (M + M.T).astype(np.float32)


LAST_RESULTS = None  # BassKernelResults of the most recent device run (for test.py)


def kernel(x, hid, gru_Wih, gru_Whh, gru_bih, gru_bhh,
           W1, b1, ln1_w, ln1_b, W2, b2, ln2_w, ln2_b,
           W3, b3, ln3_w, ln3_b, W4, b4):
    global LAST_RESULTS
    args = [np.asarray(a, np.float32) for a in
            (x, hid, gru_Wih, gru_Whh, gru_bih, gru_bhh, W1, b1, W2, b2,
             W3, b3, W4, b4)]
    trivial_ln = all(np.all(w == 1.0) for w in (ln1_w, ln2_w, ln3_w)) and \
        all(np.all(b == 0.0) for b in (ln1_b, ln2_b, ln3_b))
    if not trivial_ln:
        return _numpy_fallback(x, hid, gru_Wih, gru_Whh, gru_bih, gru_bhh,
                               W1, b1, ln1_w, ln1_b, W2, b2, ln2_w, ln2_b,
                               W3, b3, ln3_w, ln3_b, W4, b4)

    if "nc" not in _CACHE:
        _CACHE["nc"] = _build()
    nc = _CACHE["nc"]
    in_maps, iu, ju = _prep(*args)
    res = run_bass_kernel_spmd(nc, in_maps, core_ids=list(range(NCORES)),
                               trace=False)
    LAST_RESULTS = res

    W4f, b4f = args[12], args[13]
    c4 = float(W4f.sum())
    z4 = np.concatenate([res.results[c]["oz4"][0] for c in range(NCORES)])[:P]
    nm = 8
    st3 = np.stack([res.results[c]["ost3"] for c in range(NCORES)])  # [8,128,32]
    tots = st3.reshape(NCORES, 128, 4, nm).sum(axis=(1, 3))          # [8,4]
    S3 = tots[:, 0].sum() - tots[NCORES - 1, 2]
    Q3 = tots[:, 1].sum() - tots[NCORES - 1, 3]
    cnt = float(P) * H2
    mu3 = S3 / cnt
    var3 = Q3 / cnt - mu3 * mu3
    s3 = 1.0 / np.sqrt(var3 + EPS)
    v = 1.0 / (1.0 + np.exp(-(s3 * z4 + (float(b4f[0]) - s3 * mu3 * c4))))
    M = np.zeros((N_NODES, N_NODES), np.float32)
    M[iu, ju] = v.astype(np.float32)
    return (M + M.T).astype(np.float32)

